# revision 1
# baseline (speedup 1.0000x reference)
"""GRU decoder kernel for Trainium2 (8 NeuronCores, SPMD).

Problem: nn_Decoder (B=16, T=250, E=512, H=1024, V=32000)
  x      = emb_table[token_ids]                  [B,T,E]
  x_proj = x @ W + b[0]                          [B,T,3H]
  hs     = GRU scan (reset_after) over T         [B,T,H]
  logits = hs @ Wo + bo                          [B,T,V]

Strategy (per core, SPMD x8):
  - Output projection is vocab-sharded 8 ways (core c owns V-cols
    [4000c, 4000(c+1))); embedding gather happens on host; the input
    projection and the (inherently serial) GRU scan are replicated on
    every core since the scan is U-streaming bound regardless.
  - Token index is tok = t*16 + b everywhere.
  - Phase A: x_projT = W^T @ x^T computed W-stationary so results come
    out in a "packed" layout keyed (p, kc*16+b) per gate; scattered to
    DRAM xpk[t][128, 3*128].
  - Phase B: 250 sequential GRU steps. rec = h @ U via M=16 matmuls
    (lhsT = packed h slices, rhs = U rows, fp32 data read as float32r),
    then PE-transposes repack rec into the packed layout where all the
    gate elementwise/activation work runs on 128 partitions (short
    serial chain). h state is kept packed: h~[p, kc*16+b] = h[b, kc*128+p].
  - Phase C: logits = hs @ Wo_c in bf16, tok-chunk x v-chunk tiled,
    contiguous 2KB row writes into the [4000, 4000] output.
"""

import sys
import os

sys.path.insert(0, "/opt/trn_rl_repo")

import numpy as np
import ml_dtypes

import concourse.bass as bass
import concourse.mybir as mybir
from concourse import bacc
from concourse.tile import TileContext
from concourse.bass_utils import run_bass_kernel_spmd
from concourse.masks import make_identity

B, T, E, H, V = 16, 250, 512, 1024, 32000
NCORES = 8
VS = V // NCORES          # vocab shard per core = 4000
G3 = 3 * H                # 3072
NTOK = B * T              # 4000 tokens, tok = t*16 + b
KC = H // 128             # 8 h-chunks
EC = E // 128             # 4 e-chunks
MC = G3 // 128            # 24 m-chunks of x_projT

F32 = mybir.dt.float32
F32R = mybir.dt.float32r
BF16 = mybir.dt.bfloat16
AF = mybir.ActivationFunctionType


def build_program(T_steps=T, use_b1h=False, skip_proj=False, skip_phA=False):
    nc = bacc.Bacc("TRN2", target_bir_lowering=False, debug=False,
                   num_devices=NCORES)

    ntok = B * T_steps

    # ---- kernel I/O (per-core) ----
    xT_d = nc.dram_tensor("xT", [E, ntok], F32R, kind="ExternalInput").ap()
    w_d = nc.dram_tensor("W", [E, G3], F32R, kind="ExternalInput").ap()
    u_d = nc.dram_tensor("U", [H, G3], F32R, kind="ExternalInput").ap()
    bA_d = nc.dram_tensor("bA", [1, G3], F32R, kind="ExternalInput").ap()
    h0_d = nc.dram_tensor("h0pk", [128, 128], F32R, kind="ExternalInput").ap()
    ones_d = nc.dram_tensor("onesv", [1, 512], F32R, kind="ExternalInput").ap()
    wo_d = nc.dram_tensor("Wo", [H, VS], BF16, kind="ExternalInput").ap()
    b1h_d = None
    if use_b1h:
        b1h_d = nc.dram_tensor("b1h", [1, H], F32R, kind="ExternalInput").ap()

    out_d = nc.dram_tensor("logits", [ntok, VS], F32, kind="ExternalOutput").ap()
    hs_out_d = nc.dram_tensor("hs_dump", [T_steps, 128, 128], BF16,
                              kind="ExternalOutput").ap()

    # ---- internal DRAM ----
    # packed x_proj: xpk[t][p][g*128 + kc*16 + b] = x_proj[tok(t,b), g*H + kc*128 + p]
    xpk_d = nc.dram_tensor("xpk", [T_steps, 128, 3 * 128], F32).ap()

    with TileContext(nc) as tc:
        with tc.tile_pool(name="consts", bufs=1) as consts:
            ident = consts.tile([16, 16], F32)
            make_identity(nc, ident)
            ones = consts.tile([1, 512], F32R)
            nc.sync.dma_start(out=ones, in_=ones_d)

            # =========================================================
            # Phase A: x_projT (+ bias) -> packed DRAM
            # =========================================================
            with tc.tile_pool(name="phA", bufs=1) as phA, \
                 tc.tile_pool(name="phA_st", bufs=6) as phA_st, \
                 tc.tile_pool(name="phA_ps", bufs=4, space="PSUM") as phA_ps:
                w_sb = phA.tile([128, EC, G3], F32R)
                nc.sync.dma_start(
                    out=w_sb, in_=w_d.rearrange("(kc p) n -> p kc n", p=128))
                xT_sb = phA.tile([128, EC, ntok], F32R)
                nc.sync.dma_start(
                    out=xT_sb, in_=xT_d.rearrange("(kc p) t -> p kc t", p=128))
                bA_sb = phA.tile([1, G3], F32R)
                nc.sync.dma_start(out=bA_sb, in_=bA_d)

                tg = 0 if not skip_phA else 10**9
                while tg * 512 < ntok:
                    tok0 = tg * 512
                    ncols = min(512, ntok - tok0)
                    nt = ncols // 16
                    t0 = tok0 // 16
                    for m in range(MC):
                        g, kc = divmod(m, KC)
                        ps = phA_ps.tile([128, 512], F32)
                        for ec in range(EC):
                            nc.tensor.matmul(
                                ps[:, :ncols],
                                w_sb[:, ec, m * 128:(m + 1) * 128],
                                xT_sb[:, ec, tok0:tok0 + ncols],
                                start=(ec == 0), stop=False)
                        # + bias row (b[0] with b[1] z/r folded in)
                        nc.tensor.matmul(
                            ps[:, :ncols],
                            bA_sb[:, m * 128:(m + 1) * 128],
                            ones[:, :ncols],
                            start=False, stop=True)
                        st = phA_st.tile([128, 512], F32)
                        if m % 2 == 0:
                            nc.vector.tensor_copy(st[:, :ncols], ps[:, :ncols])
                        else:
                            nc.scalar.copy(st[:, :ncols], ps[:, :ncols])
                        base = g * 128 + kc * 16
                        dst = xpk_d[t0:t0 + nt, :, base:base + 16] \
                            .rearrange("t p b -> p t b")
                        nc.sync.dma_start(
                            out=dst,
                            in_=st[:, :ncols].rearrange("p (t b) -> p t b", b=16))
                    tg += 1

            # =========================================================
            # Phase B: GRU scan
            # =========================================================
            with tc.tile_pool(name="u", bufs=1) as u_pool, \
                 tc.tile_pool(name="xpk", bufs=2) as xpk_pool, \
                 tc.tile_pool(name="state", bufs=2) as state_pool, \
                 tc.tile_pool(name="recsb", bufs=2) as recsb_pool, \
                 tc.tile_pool(name="gates", bufs=2) as gates_pool, \
                 tc.tile_pool(name="hsout", bufs=4) as hsout_pool, \
                 tc.tile_pool(name="ps_rec", bufs=1, space="PSUM") as ps_rec_pool, \
                 tc.tile_pool(name="ps_pk", bufs=1, space="PSUM") as ps_pk_pool:

                u_sb = u_pool.tile([128, KC, G3], F32R)
                nc.sync.dma_start(
                    out=u_sb, in_=u_d.rearrange("(kc p) n -> p kc n", p=128))
                b1h_sb = None
                if use_b1h:
                    b1h_sb = u_pool.tile([1, H], F32R)
                    nc.sync.dma_start(out=b1h_sb, in_=b1h_d)

                h_cur = state_pool.tile([128, 128], F32R, tag="h")
                nc.sync.dma_start(out=h_cur, in_=h0_d)

                PF = 8  # xpk prefetch block (steps)
                xpk_tiles = {}

                def load_xpk_block(k):
                    t0 = k * PF
                    if t0 >= T_steps or k in xpk_tiles:
                        return
                    npf = min(PF, T_steps - t0)
                    xt = xpk_pool.tile([128, PF, 3 * 128], F32, tag="xpk")
                    nc.sync.dma_start(
                        out=xt[:, :npf, :],
                        in_=xpk_d[t0:t0 + npf].rearrange("t p c -> p t c"))
                    xpk_tiles[k] = xt

                load_xpk_block(0)
                for t in range(T_steps):
                    if t % PF == 0:
                        load_xpk_block(t // PF + 1)  # prefetch next block
                    xt = xpk_tiles[t // PF]
                    tp = t % PF

                    # --- rec = h @ U  (+ b1h), [16, 3072] in PSUM ---
                    rec_ps = ps_rec_pool.tile([16, G3], F32, tag="rec")
                    for n in range(6):
                        h_gate = use_b1h and n >= 4
                        for kc in range(KC):
                            last = (kc == KC - 1) and not h_gate
                            nc.tensor.matmul(
                                rec_ps[:, n * 512:(n + 1) * 512],
                                h_cur[:, kc * 16:(kc + 1) * 16],
                                u_sb[:, kc, n * 512:(n + 1) * 512],
                                start=(kc == 0), stop=last)
                        if h_gate:
                            nc.tensor.matmul(
                                rec_ps[:, n * 512:(n + 1) * 512],
                                b1h_sb[:, (n - 4) * 512:(n - 3) * 512],
                                ones[:, :512],
                                start=False, stop=True)

                    # --- evacuate rec to SBUF (split DVE / ACT) ---
                    rec_sb = recsb_pool.tile([16, G3], F32, tag="recsb")
                    nc.vector.tensor_copy(rec_sb[:, 0:2048], rec_ps[:, 0:2048])
                    nc.scalar.copy(rec_sb[:, 2048:2560], rec_ps[:, 2048:2560])
                    nc.vector.tensor_copy(rec_sb[:, 2560:3072],
                                          rec_ps[:, 2560:3072])

                    # --- PE transpose into packed layout ---
                    zr_pk = ps_pk_pool.tile([128, 256], F32, tag="zrpk")
                    rh_pk = ps_pk_pool.tile([128, 128], F32, tag="rhpk")
                    for g in range(2):  # z, r
                        for kc in range(KC):
                            col = g * H + kc * 128
                            nc.tensor.transpose(
                                zr_pk[:, g * 128 + kc * 16: g * 128 + kc * 16 + 16],
                                rec_sb[:, col:col + 128],
                                ident)
                    for kc in range(KC):  # rh
                        col = 2 * H + kc * 128
                        nc.tensor.transpose(
                            rh_pk[:, kc * 16:kc * 16 + 16],
                            rec_sb[:, col:col + 128],
                            ident)

                    # --- gates (packed layout, 128 partitions) ---
                    zr_arg = gates_pool.tile([128, 256], F32, tag="zrarg")
                    nc.vector.tensor_add(zr_arg, zr_pk, xt[:, tp, 0:256])
                    zr_sig = gates_pool.tile([128, 256], F32, tag="zrsig")
                    nc.scalar.activation(zr_sig, zr_arg, AF.Sigmoid)
                    z_sig = zr_sig[:, 0:128]
                    r_sig = zr_sig[:, 128:256]

                    harg = gates_pool.tile([128, 128], F32, tag="harg")
                    nc.vector.tensor_mul(harg, r_sig, rh_pk)
                    nc.vector.tensor_add(harg, harg, xt[:, tp, 256:384])
                    hh = gates_pool.tile([128, 128], F32, tag="hh")
                    nc.scalar.activation(hh, harg, AF.Tanh)

                    # h_new = z*h + (1-z)*hh  ==  z*h - (z-1)*hh
                    m1 = gates_pool.tile([128, 128], F32, tag="m1")
                    nc.vector.tensor_mul(m1, z_sig, h_cur)
                    m2 = gates_pool.tile([128, 128], F32, tag="m2")
                    nc.vector.scalar_tensor_tensor(
                        m2, z_sig, 1.0, hh,
                        op0=mybir.AluOpType.subtract, op1=mybir.AluOpType.mult)
                    h_new = state_pool.tile([128, 128], F32R, tag="h")
                    nc.vector.tensor_sub(h_new, m1, m2)

                    # --- store packed h (bf16) for the projection ---
                    h_bf = hsout_pool.tile([128, 128], BF16, tag="hbf")
                    nc.scalar.copy(h_bf, h_new)
                    nc.sync.dma_start(out=hs_out_d[t], in_=h_bf)

                    h_cur = h_new
                    if t % PF == PF - 1:
                        xpk_tiles.pop(t // PF, None)

            # =========================================================
            # Phase C: logits = hs @ Wo (bf16)
            # =========================================================
            if skip_proj:
                pass
            else:
              with tc.tile_pool(name="hsres", bufs=1) as hsres_pool, \
                 tc.tile_pool(name="wo", bufs=2) as wo_pool, \
                 tc.tile_pool(name="stC", bufs=6) as stC_pool, \
                 tc.tile_pool(name="ps_c", bufs=4, space="PSUM") as ps_c_pool:
                # hs resident, per-kc layout: hs_res[p, kc, t*16+b]
                hs_res = hsres_pool.tile([128, KC, ntok], BF16)
                nc.sync.dma_start(
                    out=hs_res.rearrange("p kc (t b) -> p kc t b", b=16),
                    in_=hs_out_d.rearrange("t p (kc b) -> p kc t b", b=16))

                n_tc = (ntok + 127) // 128
                n_vc = (VS + 511) // 512
                for vc in range(n_vc):
                    v0 = vc * 512
                    nv = min(512, VS - v0)
                    wo_sb = wo_pool.tile([128, KC, 512], BF16, tag="wo")
                    nc.sync.dma_start(
                        out=wo_sb[:, :, :nv],
                        in_=wo_d[:, v0:v0 + nv].rearrange(
                            "(kc p) v -> p kc v", p=128))
                    for tcn in range(n_tc):
                        tok0 = tcn * 128
                        ntk = min(128, ntok - tok0)
                        ps = ps_c_pool.tile([128, 512], F32, tag="cps")
                        for kc in range(KC):
                            nc.tensor.matmul(
                                ps[:ntk, :nv],
                                hs_res[:, kc, tok0:tok0 + ntk],
                                wo_sb[:, kc, :nv],
                                start=(kc == 0), stop=(kc == KC - 1))
                        st = stC_pool.tile([128, 512], F32, tag="cst")
                        if tcn % 2 == 0:
                            nc.vector.tensor_copy(st[:ntk, :nv], ps[:ntk, :nv])
                        else:
                            nc.scalar.copy(st[:ntk, :nv], ps[:ntk, :nv])
                        nc.sync.dma_start(out=out_d[tok0:tok0 + ntk, v0:v0 + nv],
                                          in_=st[:ntk, :nv])

    nc.compile()
    return nc


_prog_cache = {}


def _get_program(T_steps, use_b1h):
    key = (T_steps, use_b1h)
    if key not in _prog_cache:
        _prog_cache[key] = build_program(T_steps, use_b1h)
    return _prog_cache[key]


def kernel(token_ids, initial_state, emb_table, W, U, b, Wo, bo,
           T_steps=None, _debug=False):
    token_ids = np.asarray(token_ids)
    initial_state = np.asarray(initial_state, dtype=np.float32)
    emb_table = np.asarray(emb_table, dtype=np.float32)
    W = np.asarray(W, dtype=np.float32)
    U = np.asarray(U, dtype=np.float32)
    b = np.asarray(b, dtype=np.float32)
    Wo = np.asarray(Wo, dtype=np.float32)
    bo = np.asarray(bo, dtype=np.float32)

    Tn = token_ids.shape[1] if T_steps is None else T_steps
    ntok = B * Tn

    use_b1h = bool(np.any(b[1, 2 * H:]))
    nc = _get_program(Tn, use_b1h)

    # ---- host-side input prep ----
    x = emb_table[token_ids[:, :Tn]]                  # [B,Tn,E]
    xT = np.ascontiguousarray(x.transpose(2, 1, 0).reshape(E, ntok))
    bA = b[0].copy()
    bA[:2 * H] += b[1, :2 * H]
    bA = bA.reshape(1, G3)
    h0pk = np.ascontiguousarray(
        initial_state.reshape(B, KC, 128).transpose(2, 1, 0).reshape(128, 128))
    Wo_bf = Wo.astype(ml_dtypes.bfloat16)

    base = {
        "xT": xT, "W": np.ascontiguousarray(W),
        "U": np.ascontiguousarray(U), "bA": bA, "h0pk": h0pk,
        "onesv": np.ones((1, 512), np.float32),
    }
    if use_b1h:
        base["b1h"] = b[1, 2 * H:].reshape(1, H).copy()

    in_maps = []
    for c in range(NCORES):
        m = dict(base)
        m["Wo"] = np.ascontiguousarray(Wo_bf[:, c * VS:(c + 1) * VS])
        in_maps.append(m)

    res = run_bass_kernel_spmd(nc, in_maps, list(range(NCORES)))

    # ---- assemble [B, Tn, V] ----
    out = np.empty((B, Tn, V), np.float32)
    for c in range(NCORES):
        lg = res.results[c]["logits"].reshape(Tn, B, VS)
        out[:, :, c * VS:(c + 1) * VS] = lg.transpose(1, 0, 2)
    if np.any(bo):
        out += bo
    if _debug:
        # hs_dump[t][p][kc*16+b] -> hs [B, Tn, H] (from core 0)
        hpk = np.asarray(res.results[0]["hs_dump"], dtype=np.float32)
        hs = hpk.reshape(Tn, 128, KC, B).transpose(3, 0, 2, 1).reshape(B, Tn, H)
        return out, hs
    return out

